# revision 10
# baseline (speedup 1.0000x reference)
"""Trainium2 Bass kernel v2 for the 2-layer GAT, 8 NeuronCores.

Architecture (v2, instruction-count-driven):
  - Phase A src-sharded (7500 nodes/core): fi/ti/alpha gates feat-major,
    z|s projection node-major -> table rows [z 512 | s_src 8 | s_dst 8 | pad]
    (1280B bf16), no PE transposes.
  - ONE AllGather replicates the full node table [60000, 640] to every core.
  - Layer-1 aggregation dst-sharded: core c owns ~30 dst tiles; per-edge z
    rows gathered locally (lo/hi table views to stay in int16), s_dst rows
    gathered by dst from the same table (256B view). One-hot matmuls
    accumulate exp-weighted z + den per dst tile in PSUM; normalize + ELU +
    z2/s2 projection fused per tile -> local ztab2 rows
    [z2 128 | s2src 1 | s2dst 1 | pad] (512B).
  - ONE AllGather replicates ztab2 [8*3840, 256].
  - Layer-2 aggregation dst-sharded (8 tiles/core), fused single-pass
    matmul [zw|exp] (129 cols), normalize in psum -> out rows directly.
  - No ReduceScatter anywhere; host concatenates per-core dst ranges.
"""
import os
import sys

sys.path.insert(0, "/opt/trn_rl_repo")

import numpy as np
import ml_dtypes

import concourse.bass as bass
import concourse.bacc as bacc
import concourse.tile as tile
import concourse.mybir as mybir
from concourse.bass_utils import run_bass_kernel_spmd

BF16 = ml_dtypes.bfloat16
NCORE = 8
F = 512
H = 8
O = F // H
OUT = 128
ROW1 = 640       # [z 512 | s_src 8 | s_dst 8 | pad]  (1280B)
SROW1 = 128      # s-view row: cols 512:640 of ROW1   (256B)
ROW2 = 256       # [z2 128 | s2src 1 | s2dst 1 | pad] (512B)
SROW2 = 128      # s-view row: cols 128:256 of ROW2   (256B)
LO = 32768       # int16 split for the 60000-row table

RING = int(os.environ.get("GAT_RING", "16384"))        # SWDGE carveout bytes
CALLCH = int(os.environ.get("GAT_CALLCH", "24"))       # chunks per gather call
STAGE = int(os.environ.get("GAT_STAGE", "9"))
GB_ENV = int(os.environ.get("GAT_GB", "8"))


def _ceil(a, b):
    return -(-a // b)


def _wrap_idx(idx):
    """[S] int -> [128, S//16] int16 (wrapped in 16 partitions, tiled 8x)."""
    w = idx.reshape(-1, 16).T.astype(np.int16)
    return np.ascontiguousarray(np.tile(w, (8, 1)))


def _tile_split(n_tiles, ncore):
    """Contiguous split of n_tiles over ncore cores; first cores get +1."""
    base = n_tiles // ncore
    extra = n_tiles - base * ncore
    counts = [base + (1 if c < extra else 0) for c in range(ncore)]
    starts = np.concatenate([[0], np.cumsum(counts)]).astype(np.int64)
    return counts, starts


def _sched_dst(src, dst, n_dst, n_src_lo, tt, starts, split_lo):
    """Dst-sharded schedule.

    Slots per core: for each owned tile-slot 0..tt-1:
      [lo chunks CHL*128 | hi chunks CHH*128]  (hi empty if split_lo is None)
    Returns per-core (g, sd, dc) plus CHL, CHH.
    g: gather idx into (lo|hi) table view; sd: dst idx (for s-gather);
    dc: dst col within tile (-1 pad).
    """
    T = _ceil(n_dst, 128)
    tidx = dst // 128
    # per (core, tileslot) edge lists
    per = {}
    chl_max, chh_max = 1, (1 if split_lo is not None else 0)
    for c in range(NCORE):
        for ts in range(tt):
            gt = starts[c] + ts
            empty = (np.zeros(0, np.int64), np.zeros(0, np.int64))
            if gt >= starts[c + 1]:
                per[(c, ts)] = (empty, empty)
                continue
            m = tidx == gt
            s, d = src[m], dst[m]
            if split_lo is not None:
                mlo = s < split_lo
                e_lo = (s[mlo], d[mlo])
                e_hi = (s[~mlo] - split_lo, d[~mlo])
                chl_max = max(chl_max, _ceil(len(e_lo[0]), 128))
                chh_max = max(chh_max, _ceil(len(e_hi[0]), 128))
                per[(c, ts)] = (e_lo, e_hi)
            else:
                chl_max = max(chl_max, _ceil(len(s), 128))
                per[(c, ts)] = ((s, d), (np.zeros(0, np.int64),
                                         np.zeros(0, np.int64)))
    CHL, CHH = chl_max, chh_max
    PT = (CHL + CHH) * 128
    S = tt * PT
    g = np.zeros((NCORE, S), np.int64)
    sd = np.zeros((NCORE, S), np.int64)
    dc = np.full((NCORE, S), -1.0, np.float32)
    for c in range(NCORE):
        for ts in range(tt):
            gt = starts[c] + ts
            base = ts * PT
            (slo, dlo), (shi, dhi) = per[(c, ts)]
            for off, (ss, dd) in ((0, (slo, dlo)), (CHL * 128, (shi, dhi))):
                n = len(ss)
                if n == 0:
                    continue
                g[c, base + off:base + off + n] = ss
                sd[c, base + off:base + off + n] = dd
                dc[c, base + off:base + off + n] = (
                    dd - gt * 128).astype(np.float32)
    return g, sd, dc, CHL, CHH


def _prep(inputs):
    img = np.asarray(inputs["image_features"], np.float32)
    blk = np.asarray(inputs["block_features"], np.float32)
    W_img = np.asarray(inputs["W_img"], np.float32)
    W_blk = np.asarray(inputs["W_blk"], np.float32)
    Wv = np.asarray(inputs["Wv"], np.float32)
    bv = np.asarray(inputs["bv"], np.float32)
    We = np.asarray(inputs["We"], np.float32)
    be = np.asarray(inputs["be"], np.float32)
    fc1 = np.asarray(inputs["fc1"], np.float32)
    attn1 = np.asarray(inputs["attn1"], np.float32)
    fc2 = np.asarray(inputs["fc2"], np.float32)
    attn2 = np.asarray(inputs["attn2"], np.float32)
    e0s = np.asarray(inputs["edge0_src"], np.int64)
    e0d = np.asarray(inputs["edge0_dst"], np.int64)
    e1s = np.asarray(inputs["edge1_src"], np.int64)
    e1d = np.asarray(inputs["edge1_dst"], np.int64)
    ND0 = int(inputs["n_dst0"])
    ND1 = int(inputs["n_dst1"])
    NS, IMG = img.shape
    BLK = blk.shape[1]
    SS = NS // NCORE

    T1 = _ceil(ND0, 128)            # 235
    T2 = _ceil(ND1, 128)            # 63
    cnt1, st1 = _tile_split(T1, NCORE)
    cnt2, st2 = _tile_split(T2, NCORE)
    TT1, TT2 = max(cnt1), max(cnt2)

    # L2 src remap: L1-dst node d -> padded global ztab2 row
    owner1 = np.searchsorted(st1, (np.arange(ND0) // 128), side="right") - 1
    remap = owner1 * (TT1 * 128) + (np.arange(ND0) - st1[owner1] * 128)
    e1s_r = remap[e1s]
    assert e1s_r.max() < NCORE * TT1 * 128 <= 32768

    g1, sd1, dc1, CHL1, CHH1 = _sched_dst(e0s, e0d, ND0, LO, TT1, st1, LO)
    g2, sd2r, dc2, CHL2, _ = _sched_dst(e1s_r, e1d, ND1, None, TT2, st2, None)
    # sd2 must index ztab2 rows too (s2dst lives in dst node's row)
    sd2 = np.zeros_like(sd2r)
    m = dc2.reshape(NCORE, -1) >= -0.5
    sd2[m] = remap[sd2r[m]]

    # host-derived weights
    wimgT = np.ascontiguousarray(W_img.T).astype(BF16)
    wblkT = np.ascontiguousarray(W_blk.T).astype(BF16)
    wv = Wv.astype(BF16)
    we = We.astype(BF16)
    # z|s projection: [512 -> 528]: [fc1 (h o) | a_src 8 | a_dst 8]
    fc1f = fc1.reshape(F, F)                        # [(h o), F]
    a_src = np.einsum("hof,ho->fh", fc1, attn1[:, :O])
    a_dst = np.einsum("hof,ho->fh", fc1, attn1[:, O:])
    zsw = np.concatenate([fc1f.T, a_src, a_dst], axis=1).astype(BF16)  # [F,528]
    # z2|s2 projection: [512 -> 130]
    f2T = fc2[0].T                                   # [F, OUT]
    w2s = f2T @ attn2[0, :OUT]
    w2d = f2T @ attn2[0, OUT:]
    fc2cat = np.concatenate([f2T, w2s[:, None], w2d[:, None]],
                            axis=1).astype(BF16)     # [F, 130]
    MB = F // 128
    biasv = np.ascontiguousarray(bv.reshape(MB, 128).T).astype(np.float32)
    biase = np.ascontiguousarray(be.reshape(MB, 128).T).astype(np.float32)
    iota = np.tile(np.arange(128, dtype=np.float32), (128, 1))
    ident = np.eye(128, dtype=np.float32).astype(BF16)

    S1 = TT1 * (CHL1 + CHH1) * 128
    S2 = TT2 * CHL2 * 128

    shared = dict(wimgT=wimgT, wblkT=wblkT, wv=wv, we=we, zsw=zsw,
                  fc2cat=fc2cat, biasv=biasv, biase=biase, iota=iota,
                  ident=ident, tick=np.zeros((128, 1), np.float32))
    in_maps = []
    for c in range(NCORE):
        m = dict(shared)
        m["imgT"] = np.ascontiguousarray(img[c * SS:(c + 1) * SS].T).astype(BF16)
        m["blkT"] = np.ascontiguousarray(blk[c * SS:(c + 1) * SS].T).astype(BF16)
        m["g1"] = _wrap_idx(g1[c])
        m["sd1"] = _wrap_idx(sd1[c])
        m["dc1"] = np.ascontiguousarray(dc1[c].reshape(-1, 128).T)
        m["g2"] = _wrap_idx(g2[c])
        m["sd2"] = _wrap_idx(sd2[c])
        m["dc2"] = np.ascontiguousarray(dc2[c].reshape(-1, 128).T)
        in_maps.append(m)

    cfg = dict(NS=NS, IMG=IMG, BLK=BLK, ND0=ND0, ND1=ND1, SS=SS,
               TT1=TT1, TT2=TT2, CHL1=CHL1, CHH1=CHH1, CHL2=CHL2,
               S1=S1, S2=S2)
    return cfg, in_maps, (cnt2, st2)


# ---------------------------------------------------------------- device code

def _build(cfg):
    REPEAT = cfg.get("repeat", 1)
    bf16 = mybir.dt.bfloat16
    f32 = mybir.dt.float32
    i16 = mybir.dt.int16
    NS, IMG, BLK, SS = cfg["NS"], cfg["IMG"], cfg["BLK"], cfg["SS"]
    ND0, ND1 = cfg["ND0"], cfg["ND1"]
    TT1, TT2 = cfg["TT1"], cfg["TT2"]
    CHL1, CHH1, CHL2 = cfg["CHL1"], cfg["CHH1"], cfg["CHL2"]
    S1, S2 = cfg["S1"], cfg["S2"]
    KI, KB, MB = IMG // 128, BLK // 128, F // 128
    PT1 = (CHL1 + CHH1) * 128
    NROW2 = NCORE * TT1 * 128          # global ztab2 rows (30720)
    Sig = mybir.ActivationFunctionType.Sigmoid
    Exp = mybir.ActivationFunctionType.Exp
    MUL = mybir.AluOpType.mult
    ADD = mybir.AluOpType.add
    EQ = mybir.AluOpType.is_equal

    nc = bacc.Bacc("TRN2", target_bir_lowering=False, debug=False,
                   num_devices=NCORE, dynamic_dma_scratch_size=RING)
    TT = nc.vector.tensor_tensor

    def param(name, shape, dt):
        return nc.declare_dram_parameter(name, list(shape), dt, isOutput=False)

    imgT = param("imgT", [IMG, SS], bf16)
    blkT = param("blkT", [BLK, SS], bf16)
    wimgT = param("wimgT", [IMG, F], bf16)
    wblkT = param("wblkT", [BLK, F], bf16)
    wv = param("wv", [F, F], bf16)
    we = param("we", [F, F], bf16)
    zsw = param("zsw", [F, F + 16], bf16)
    fc2cat = param("fc2cat", [F, OUT + 2], bf16)
    biasv = param("biasv", [128, MB], f32)
    biase = param("biase", [128, MB], f32)
    iota = param("iota", [128, 128], f32)
    ident = param("ident", [128, 128], bf16)
    g1 = param("g1", [128, S1 // 16], i16)
    sd1 = param("sd1", [128, S1 // 16], i16)
    dc1 = param("dc1", [128, S1 // 128], f32)
    g2 = param("g2", [128, S2 // 16], i16)
    sd2 = param("sd2", [128, S2 // 16], i16)
    dc2 = param("dc2", [128, S2 // 128], f32)
    tick = param("tick", [128, 1], f32)
    out = nc.declare_dram_parameter("out", [TT2 * 128, OUT], f32,
                                    isOutput=True)
    tock = nc.declare_dram_parameter("tock", [128, 1], f32, isOutput=True)

    NB = min(3, REPEAT)
    ztab1l_ = [nc.dram_tensor(f"ztab1l{p}", [SS, ROW1], bf16)
               for p in range(NB)]
    ztab1_ = [nc.dram_tensor(f"ztab1_{p}", [NS, ROW1], bf16,
                             addr_space="Shared") for p in range(NB)]
    ztab1b_ = [nc.dram_tensor(f"ztab1b{p}", [NS, ROW1], bf16)
               for p in range(NB)]
    ztab2l_ = [nc.dram_tensor(f"ztab2l{p}", [TT1 * 128, ROW2], bf16)
               for p in range(NB)]
    ztab2_ = [nc.dram_tensor(f"ztab2_{p}", [NROW2, ROW2], bf16,
                             addr_space="Shared") for p in range(NB)]
    ztab2b_ = [nc.dram_tensor(f"ztab2b{p}", [NROW2, ROW2], bf16)
               for p in range(NB)]

    from contextlib import ExitStack
    with tile.TileContext(nc) as tc, ExitStack() as top:
        res = top.enter_context(tc.tile_pool(name="res", bufs=1))

        def resident(p, k, m):
            t = res.tile([128, k * m], bf16, name=p.name + "_sb")
            nc.sync.dma_start(t[:].rearrange("p (k m) -> p k m", k=k),
                              p[:, :].rearrange("(k p) m -> p k m", p=128))
            return t

        wimg_sb = resident(wimgT, KI, F)
        wblk_sb = resident(wblkT, KB, F)
        wv_sb = resident(wv, MB, F)
        we_sb = resident(we, MB, F)
        zsw_sb = resident(zsw, MB, F + 16)
        fc2_sb = resident(fc2cat, MB, OUT + 2)
        bv_sb = res.tile([128, MB], f32)
        nc.sync.dma_start(bv_sb[:], biasv[:, :])
        be_sb = res.tile([128, MB], f32)
        nc.sync.dma_start(be_sb[:], biase[:, :])
        iota_sb = res.tile([128, 128], f32)
        nc.sync.dma_start(iota_sb[:], iota[:, :])
        id_sb = res.tile([128, 128], bf16)
        nc.sync.dma_start(id_sb[:], ident[:, :])
        g1_sb = res.tile([128, S1 // 16], i16)
        nc.sync.dma_start(g1_sb[:], g1[:, :])
        sd1_sb = res.tile([128, S1 // 16], i16)
        nc.sync.dma_start(sd1_sb[:], sd1[:, :])
        dc1_sb = res.tile([128, S1 // 128], f32)
        nc.sync.dma_start(dc1_sb[:], dc1[:, :])
        g2_sb = res.tile([128, S2 // 16], i16)
        nc.sync.dma_start(g2_sb[:], g2[:, :])
        sd2_sb = res.tile([128, S2 // 16], i16)
        nc.sync.dma_start(sd2_sb[:], sd2[:, :])
        dc2_sb = res.tile([128, S2 // 128], f32)
        nc.sync.dma_start(dc2_sb[:], dc2[:, :])
        tk = res.tile([128, 1], f32)
        nc.sync.dma_start(tk[:], tick[:, :])
        nc.sync.dma_start(tock[:, :], tk[:])

        for _rep in range(REPEAT):
            _pb = _rep % NB
            ztab1l, ztab1, ztab1b = ztab1l_[_pb], ztab1_[_pb], ztab1b_[_pb]
            ztab2l, ztab2, ztab2b = ztab2l_[_pb], ztab2_[_pb], ztab2b_[_pb]
            # ------------- Phase A: per-node transforms + table rows -------
            WA = 512
            if STAGE >= 1:
              with ExitStack() as pa:
                rhsp = pa.enter_context(tc.tile_pool(name=f"parhs{_rep}", bufs=2))
                sbp = pa.enter_context(tc.tile_pool(name=f"pasb{_rep}", bufs=2))
                psp = pa.enter_context(tc.tile_pool(name=f"paps{_rep}", bufs=4,
                                                    space="PSUM"))
                pzs = pa.enter_context(tc.tile_pool(name=f"pazs{_rep}", bufs=2,
                                                    space="PSUM"))
                stp = pa.enter_context(tc.tile_pool(name=f"past{_rep}", bufs=2))
                for nt in range(_ceil(SS, WA)):
                    n0 = nt * WA
                    w = min(WA, SS - n0)
                    x_sb = rhsp.tile([128, KI * w], bf16, tag="x", name="x")
                    nc.sync.dma_start(
                        x_sb[:].rearrange("p (k n) -> p k n", k=KI),
                        imgT[:, n0:n0 + w].rearrange("(k p) n -> p k n", p=128))
                    b_sb = rhsp.tile([128, KB * w], bf16, tag="b", name="b")
                    nc.sync.dma_start(
                        b_sb[:].rearrange("p (k n) -> p k n", k=KB),
                        blkT[:, n0:n0 + w].rearrange("(k p) n -> p k n", p=128))

                    def mm(lhs_sb, rhs_sb, K, m, width):
                        ps = psp.tile([128, width], f32, tag="ps", name="ps")
                        for k in range(K):
                            nc.tensor.matmul(
                                ps[:],
                                lhs_sb[:, k * F + m * 128:k * F + m * 128 + 128],
                                rhs_sb[:, k * width:(k + 1) * width],
                                start=(k == 0), stop=(k == K - 1))
                        return ps

                    fi_sb = sbp.tile([128, MB * w], bf16, tag="fi", name="fi")
                    ti_sb = sbp.tile([128, MB * w], bf16, tag="ti", name="ti")
                    av_sb = sbp.tile([128, MB * w], bf16, tag="av", name="av")
                    ae_sb = sbp.tile([128, MB * w], bf16, tag="ae", name="ae")
                    for m in range(MB):
                        ps = mm(wimg_sb, x_sb, KI, m, w)
                        nc.vector.tensor_copy(fi_sb[:, m * w:(m + 1) * w], ps[:])
                    for m in range(MB):
                        ps = mm(wblk_sb, b_sb, KB, m, w)
                        nc.vector.tensor_copy(ti_sb[:, m * w:(m + 1) * w], ps[:])
                    for m in range(MB):
                        ps = mm(wv_sb, fi_sb, MB, m, w)
                        nc.scalar.activation(av_sb[:, m * w:(m + 1) * w], ps[:],
                                             Sig, bias=bv_sb[:, m:m + 1])
                    for m in range(MB):
                        ps = mm(we_sb, ti_sb, MB, m, w)
                        nc.scalar.activation(ae_sb[:, m * w:(m + 1) * w], ps[:],
                                             Sig, bias=be_sb[:, m:m + 1])
                    fu_sb = sbp.tile([128, MB * w], bf16, tag="fu", name="fu")
                    TT(fu_sb[:], av_sb[:], fi_sb[:], MUL)
                    TT(ae_sb[:], ae_sb[:], ti_sb[:], MUL)
                    TT(fu_sb[:], fu_sb[:], ae_sb[:], ADD)
                    # z|s node-major: per 128-node block, out [nodes, 528]
                    for b0 in range(0, w, 128):
                        bw = min(128, w - b0)
                        pz = pzs.tile([128, F + 16], f32, tag="pz", name="pz")
                        for k in range(MB):
                            nc.tensor.matmul(
                                pz[:bw, 0:F],
                                fu_sb[:, k * w + b0:k * w + b0 + bw],
                                zsw_sb[:, k * (F + 16):k * (F + 16) + F],
                                start=(k == 0), stop=(k == MB - 1))
                        for k in range(MB):
                            nc.tensor.matmul(
                                pz[:bw, F:F + 16],
                                fu_sb[:, k * w + b0:k * w + b0 + bw],
                                zsw_sb[:, k * (F + 16) + F:(k + 1) * (F + 16)],
                                start=(k == 0), stop=(k == MB - 1))
                        st = stp.tile([128, F + 16], bf16, tag="st", name="st")
                        nc.vector.tensor_copy(st[:bw, :], pz[:bw, :])
                        nc.sync.dma_start(
                            ztab1l[n0 + b0:n0 + b0 + bw, 0:F + 16],
                            st[:bw, :])

            if STAGE >= 2:
                nc.gpsimd.collective_compute(
                    "AllGather", mybir.AluOpType.bypass,
                    replica_groups=[list(range(NCORE))],
                    ins=[ztab1l[:, :]], outs=[ztab1[:, :]])
                nc.sync.dma_start(ztab1b[:, :], ztab1[:, :])

            # ------------- Layer-1 aggregation (dst-sharded) ---------------
            if STAGE >= 3:
              with ExitStack() as ag:
                gzp = ag.enter_context(tc.tile_pool(name=f"gz{_rep}", bufs=2))
                gsp = ag.enter_context(tc.tile_pool(name=f"gs{_rep}", bufs=2))
                ohp = ag.enter_context(tc.tile_pool(name=f"oh{_rep}", bufs=2))
                zzp = ag.enter_context(tc.tile_pool(name=f"zz{_rep}", bufs=2))
                esp = ag.enter_context(tc.tile_pool(name=f"es{_rep}", bufs=2))
                fip = ag.enter_context(tc.tile_pool(name=f"fi1_{_rep}", bufs=2))
                php = ag.enter_context(tc.tile_pool(name=f"ph1_{_rep}", bufs=2,
                                                    space="PSUM"))
                ptp = ag.enter_context(tc.tile_pool(name=f"pt1_{_rep}", bufs=2,
                                                    space="PSUM"))
                pz2p = ag.enter_context(tc.tile_pool(name=f"pz2_{_rep}", bufs=2,
                                                     space="PSUM"))

                CH1 = CHL1 + CHH1
                GB = GB_ENV  # chunks per gather call

                def gather_piecewise(dst_tile, col0, row_elems, table_ap,
                                     idx_sb, slot0, nchunks, estep=None):
                    j = 0
                    while j < nchunks:
                        nch = min(GB, nchunks - j)
                        sa = slot0 + j * 128
                        nidx = nch * 128
                        nc.gpsimd.dma_gather(
                            dst_tile[:, col0 + j * row_elems:
                                     col0 + (j + nch) * row_elems].rearrange(
                                "p (c e) -> p c e", e=row_elems),
                            table_ap, idx_sb[:, sa // 16:(sa + nidx) // 16],
                            nidx, nidx, row_elems, elem_step=estep)
                        j += nch

                for t in range(TT1):
                    s0 = t * PT1
                    s0h = s0 + CHL1 * 128
                    zt = gzp.tile([128, CH1 * ROW1], bf16, tag="zrow",
                                  name="zrow")
                    gather_piecewise(zt, 0, ROW1, ztab1b[0:LO, :], g1_sb,
                                     s0, CHL1)
                    gather_piecewise(zt, CHL1 * ROW1, ROW1, ztab1b[LO:NS, :],
                                     g1_sb, s0h, CHH1)
                    st_ = gsp.tile([128, CH1 * SROW1], bf16, tag="srow",
                                   name="srow")
                    gather_piecewise(st_, 0, SROW1, ztab1b[:, F:ROW1], sd1_sb,
                                     s0, CHL1, estep=ROW1)
                    gather_piecewise(st_, CHL1 * SROW1, SROW1,
                                     ztab1b[:, F:ROW1], sd1_sb, s0h, CHH1,
                                     estep=ROW1)
                    z3 = zt[:].rearrange("p (c e) -> p c e", e=ROW1)
                    s3 = st_[:].rearrange("p (c e) -> p c e", e=SROW1)
                    # one-hot [slot, (c d)]
                    oh = ohp.tile([128, CH1 * 128], bf16, tag="oh", name="oh")
                    TT(oh[:].rearrange("p (c d) -> p c d", d=128),
                       iota_sb[:].unsqueeze(1).broadcast_to([128, CH1, 128]),
                       dc1_sb[:, t * CH1:(t + 1) * CH1].unsqueeze(2)
                       .broadcast_to([128, CH1, 128]), EQ)
                    # e = lrelu(s_src + s_dst); exp
                    esc = esp.tile([128, CH1 * H], f32, tag="esc", name="esc")
                    e3 = esc[:].rearrange("p (c h) -> p c h", h=H)
                    TT(e3, z3[:, :, F:F + H], s3[:, :, H:2 * H], ADD)
                    nc.vector.scalar_tensor_tensor(
                        esc[:], esc[:], 0.01, esc[:], MUL, mybir.AluOpType.max)
                    exw = esp.tile([128, CH1 * H], bf16, tag="exw", name="exw")
                    nc.scalar.activation(exw[:], esc[:], Exp)
                    x3 = exw[:].rearrange("p (c h) -> p c h", h=H)
                    # zw = z * exp (per head)
                    zz = zzp.tile([128, CH1 * F], bf16, tag="zz", name="zz")
                    TT(zz[:].rearrange("p (c h o) -> p c h o", h=H, o=O),
                       z3[:, :, 0:F].rearrange("p c (h o) -> p c h o", h=H),
                       x3.unsqueeze(3).broadcast_to([128, CH1, H, O]), MUL)
                    # accumulate h then den
                    ph = php.tile([128, F], f32, tag="ph", name="ph")
                    pd = pz2p.tile([128, 2 * H], f32, tag="pd", name="pd")
                    for j in range(CH1):
                        nc.tensor.matmul(ph[:], oh[:, j * 128:(j + 1) * 128],
                                         zz[:, j * F:(j + 1) * F],
                                         start=(j == 0), stop=(j == CH1 - 1))
                    for j in range(CH1):
                        nc.tensor.matmul(pd[:, 0:H],
                                         oh[:, j * 128:(j + 1) * 128],
                                         exw[:, j * H:(j + 1) * H],
                                         start=(j == 0), stop=(j == CH1 - 1))
                    # normalize + ELU -> h1 bf16
                    rd = esp.tile([128, H], f32, tag="rd", name="rd")
                    nc.vector.reciprocal(rd[:], pd[:, 0:H])
                    h1 = fip.tile([128, F], f32, tag="h1", name="h1")
                    TT(h1[:].rearrange("p (h o) -> p h o", h=H),
                       ph[:].rearrange("p (h o) -> p h o", h=H),
                       rd[:].unsqueeze(2).broadcast_to([128, H, O]), MUL)
                    t1 = fip.tile([128, F], f32, tag="t1", name="t1")
                    nc.vector.tensor_scalar_min(t1[:], h1[:], 0.0)
                    nc.scalar.activation(t1[:], t1[:], Exp)
                    h1e = fip.tile([128, F], bf16, tag="h1e", name="h1e")
                    nc.vector.scalar_tensor_tensor(
                        h1e[:], t1[:], -1.0, h1[:], ADD, mybir.AluOpType.max)
                    # transpose h1e -> [F, 128] then z2|s2 = h1e @ fc2cat
                    ptr = ptp.tile([128, F], bf16, tag="ptr", name="ptr")
                    for mb in range(MB):
                        nc.tensor.matmul(ptr[:, mb * 128:(mb + 1) * 128],
                                         h1e[:, mb * 128:(mb + 1) * 128],
                                         id_sb[:], is_transpose=True)
                    h1t = fip.tile([128, F], bf16, tag="h1t", name="h1t")
                    nc.vector.tensor_copy(h1t[:], ptr[:])
                    pz2 = pz2p.tile([128, OUT + 2], f32, tag="pz2", name="pz2")
                    for k in range(MB):
                        nc.tensor.matmul(
                            pz2[:], h1t[:, k * 128:(k + 1) * 128],
                            fc2_sb[:, k * (OUT + 2):(k + 1) * (OUT + 2)],
                            start=(k == 0), stop=(k == MB - 1))
                    st2 = fip.tile([128, OUT + 2], bf16, tag="st2", name="st2")
                    nc.vector.tensor_copy(st2[:], pz2[:])
                    nc.sync.dma_start(
                        ztab2l[t * 128:(t + 1) * 128, 0:OUT + 2], st2[:])

            if STAGE >= 4:
                nc.gpsimd.collective_compute(
                    "AllGather", mybir.AluOpType.bypass,
                    replica_groups=[list(range(NCORE))],
                    ins=[ztab2l[:, :]], outs=[ztab2[:, :]])
                nc.sync.dma_start(ztab2b[:, :], ztab2[:, :])

            # ------------- Layer-2 aggregation (dst-sharded) ---------------
            if STAGE >= 5:
              with ExitStack() as ag2:
                gzp = ag2.enter_context(tc.tile_pool(name=f"g2z{_rep}", bufs=3))
                gsp = ag2.enter_context(tc.tile_pool(name=f"g2s{_rep}", bufs=3))
                ohp = ag2.enter_context(tc.tile_pool(name=f"oh2{_rep}", bufs=2))
                zzp = ag2.enter_context(tc.tile_pool(name=f"zz2{_rep}", bufs=2))
                esp = ag2.enter_context(tc.tile_pool(name=f"es2{_rep}", bufs=2))
                php = ag2.enter_context(tc.tile_pool(name=f"ph2{_rep}", bufs=2,
                                                     space="PSUM"))
                CH2 = CHL2
                ZCOL = OUT + 1
                GB = GB_ENV

                def gather_piecewise2(dst_tile, row_elems, table_ap, idx_sb,
                                      slot0, nchunks, estep=None):
                    j = 0
                    while j < nchunks:
                        nch = min(GB, nchunks - j)
                        sa = slot0 + j * 128
                        nidx = nch * 128
                        nc.gpsimd.dma_gather(
                            dst_tile[:, j * row_elems:
                                     (j + nch) * row_elems].rearrange(
                                "p (c e) -> p c e", e=row_elems),
                            table_ap, idx_sb[:, sa // 16:(sa + nidx) // 16],
                            nidx, nidx, row_elems, elem_step=estep)
                        j += nch

                for t in range(TT2):
                    zt = gzp.tile([128, CH2 * ROW2], bf16, tag="z2r", name="z2r")
                    s0 = t * CH2 * 128
                    gather_piecewise2(zt, ROW2, ztab2b[:, :], g2_sb, s0, CH2)
                    st_ = gsp.tile([128, CH2 * SROW2], bf16, tag="s2r",
                                   name="s2r")
                    gather_piecewise2(st_, SROW2, ztab2b[:, OUT:ROW2], sd2_sb,
                                      s0, CH2, estep=ROW2)
                    z3 = zt[:].rearrange("p (c e) -> p c e", e=ROW2)
                    s3 = st_[:].rearrange("p (c e) -> p c e", e=SROW2)
                    oh = ohp.tile([128, CH2 * 128], bf16, tag="oh2", name="oh2")
                    TT(oh[:].rearrange("p (c d) -> p c d", d=128),
                       iota_sb[:].unsqueeze(1).broadcast_to([128, CH2, 128]),
                       dc2_sb[:, t * CH2:(t + 1) * CH2].unsqueeze(2)
                       .broadcast_to([128, CH2, 128]), EQ)
                    esc = esp.tile([128, CH2], f32, tag="esc2", name="esc2")
                    TT(esc[:].unsqueeze(2), z3[:, :, OUT:OUT + 1],
                       s3[:, :, 1:2], ADD)
                    nc.vector.scalar_tensor_tensor(
                        esc[:], esc[:], 0.01, esc[:], MUL, mybir.AluOpType.max)
                    exw = esp.tile([128, CH2], bf16, tag="exw2", name="exw2")
                    nc.scalar.activation(exw[:], esc[:], Exp)
                    zz = zzp.tile([128, CH2 * ZCOL], bf16, tag="zz2",
                                  name="zz2")
                    zc = zz[:].rearrange("p (c e) -> p c e", e=ZCOL)
                    TT(zc[:, :, 0:OUT], z3[:, :, 0:OUT],
                       exw[:].unsqueeze(2).broadcast_to([128, CH2, OUT]), MUL)
                    nc.vector.tensor_copy(zc[:, :, OUT:ZCOL],
                                          exw[:].unsqueeze(2))
                    ph = php.tile([128, ZCOL], f32, tag="ph2", name="ph2")
                    for j in range(CH2):
                        nc.tensor.matmul(ph[:], oh[:, j * 128:(j + 1) * 128],
                                         zz[:, j * ZCOL:(j + 1) * ZCOL],
                                         start=(j == 0), stop=(j == CH2 - 1))
                    rd = esp.tile([128, 1], f32, tag="rd2", name="rd2")
                    nc.vector.reciprocal(rd[:], ph[:, OUT:ZCOL])
                    ov = esp.tile([128, OUT], f32, tag="ov", name="ov")
                    TT(ov[:], ph[:, 0:OUT],
                       rd[:].broadcast_to([128, OUT]), MUL)
                    nc.sync.dma_start(out[t * 128:(t + 1) * 128, :], ov[:])

    nc.compile()
    return nc


_CACHE = {}


def _get_nc(cfg):
    key = repr(sorted((k, str(v)) for k, v in cfg.items())) + f"|{STAGE}|{RING}|{GB_ENV}"
    if key not in _CACHE:
        _CACHE[key] = _build(cfg)
    return _CACHE[key]


def kernel(**inputs) -> np.ndarray:
    cfg, in_maps, (cnt2, st2) = _prep(inputs)
    nc = _get_nc(cfg)
    res = run_bass_kernel_spmd(nc, in_maps, core_ids=list(range(NCORE)))
    ND1 = cfg["ND1"]
    parts = []
    for c in range(NCORE):
        rows = cnt2[c] * 128
        parts.append(res.results[c]["out"][:rows])
    full = np.concatenate(parts, axis=0)[:ND1]
    return full.astype(np.float32)
